# revision 1
# baseline (speedup 1.0000x reference)
"""Trainium2 Bass kernel for nn_CausalMultiresConv1d (fp16 3-engine split).

Reference computation (per batch b, channel c):
    r_0 = x
    y   = sum_{lvl=0..7} w[:, 8-lvl] * (h1 *_{d=2^lvl} r_lvl)
          + w[:,0] * r_8 + w[:,9] * x,   r_{lvl+1} = h0 *_d r_lvl
    out = gelu(y)   (exact erf gelu; causal depthwise convs, K=4 taps)

Sharding: pure data parallel - 1 batch element per NeuronCore (B=8, 8 cores).
Per-core layout: [64ch, 32768] packed as [128 partitions, 768+16384]:
  partition p = 64*j + c  ->  channel c, L-half j; 768-col causal halo
  (>= 765 receptive field), so both halves compute with no communication.

Everything is fp16 in SBUF (tolerance is 2e-2; fp16 keeps total error
~0.3%). Engine split:
  - TensorE: 31 of the 37 y-taps as fp16 diagonal matmuls (1 cyc/col)
    accumulated in PSUM fp32 across level-GROUPS; each 2048-col stripe is
    evicted by ACT to an fp16 partial E; also the level-0 chain conv
    (d=1 would break DVE 2x alignment).
  - VectorE (DVE): chain convs levels 1-6 as fp16 scalar_tensor_tensor
    at 2x mode (even shifts keep 4B alignment), 6 donated y-taps
    (k=1 of levels 2-7), and 4 merge adds y += E_g. The first donated tap
    reads in1=E0 instead of y (creates y, no extra pass).
  - ScalarE (ACT): chain tap0 scaled copies, PSUM evictions (fp32->fp16),
    final exact GELU, streamed to the output DMA per stripe.
"""

import numpy as np

import concourse.bass as bass
import concourse.mybir as mybir
from concourse.bass_utils import run_bass_kernel_spmd
from concourse.tile import TileContext

# The walrus build here rejects instructions carrying more than one sync-wait
# ("Too many sync wait commands"). Tile's kernel-tail drain attaches a wait
# for every outstanding semaphore to a single SP Drain. _TC splits them.


class _TC(TileContext):
    def __exit__(self, *a):
        r = super().__exit__(*a)
        _split_multi_waits(self.nc)
        return r


def _split_multi_waits(nc):
    n = 0
    for fn in nc.m.functions:
        for blk in fn.blocks:
            insts = getattr(blk, "instructions", None)
            if insts is None:
                continue
            new = []
            for inst in insts:
                si = getattr(inst, "sync_info", None)
                waits = list(si.on_wait) if si is not None and si.on_wait else []
                if len(waits) > 1:
                    for j, wcmd in enumerate(waits[:-1]):
                        nop = mybir.InstNoOp(
                            name=f"{inst.name}-hw{j}", engine=inst.engine
                        )
                        nop.sync_info = mybir.SyncInfo(
                            on_wait=[wcmd], on_update=[]
                        )
                        new.append(nop)
                        n += 1
                    inst.sync_info = mybir.SyncInfo(
                        on_wait=[waits[-1]], on_update=list(si.on_update)
                    )
                new.append(inst)
            blk.instructions[:] = new
    return n


B, C, L = 8, 64, 32768
K, DEPTH = 4, 8
NCORES = 8
NCHUNK = 2
CL = L // NCHUNK          # 16384 columns per chunk
PAD = 768                 # halo >= 765 = total receptive field
W = PAD + CL              # 17152 buffer columns
P = NCHUNK * C            # 128 partitions
MMN = 512                 # matmul free-dim tile (one PSUM bank of fp32)
STRIPE = 2048             # eviction stripe = 4 PSUM banks
NSTR = CL // STRIPE       # 8 y stripes

# donated y-taps (level, tap k) run on DVE instead of PE; k=1 of levels
# 2..6 keeps all DVE views 4B-aligned (d even) and spreads one tap per
# group boundary so the first one can consume E0 as its in1.
DONATE = [(lvl, 1) for lvl in range(2, 7)]

F16 = mybir.dt.float16
F32 = mybir.dt.float32
MULT = mybir.AluOpType.mult
ADD = mybir.AluOpType.add

# ---- tap tables (shared between host packing and device emission) ----
# chain diag blocks 0..3: tap k weight h0[3-k]
# block 4: w9 (y += w[:,9] * x)
# then PE y-tap blocks in emission order.
_PE_Y = []                      # [(lvl_or_'w0', k)]
_PE_Y += [(0, k) for k in range(4)]
for lvl in range(1, 8):
    for k in range(4):
        if (lvl, k) not in DONATE:
            _PE_Y.append((lvl, k))
_PE_Y += [("w0", k) for k in range(4)]
NDIAG = 5 + len(_PE_Y)          # 4 chain + w9 + y-taps
XCOLS = W + NDIAG * P
# scalar table (fp32): 0..2 chain STT k=1..3 -> h0[3-k]; 3: h0[3] (tap0);
# 4..: donated tap scalars
NSC = 4 + len(DONATE)

# groups of PE y-taps: G0 reads x; G1..G4 read two chain levels each
_GROUPS = [
    [("w9", 0)] + [(0, k) for k in range(4)],
    [(l, k) for l in (1, 2) for k in range(4) if (l, k) not in DONATE],
    [(l, k) for l in (3, 4) for k in range(4) if (l, k) not in DONATE],
    [(l, k) for l in (5, 6) for k in range(4) if (l, k) not in DONATE],
    [(7, k) for k in range(4) if (7, k) not in DONATE]
    + [("w0", k) for k in range(4)],
]


def _diag_block(tap):
    """diag table block index for a PE tap."""
    if tap[0] == "w9":
        return 4
    return 5 + _PE_Y.index(tap)


def _build_nc(reps=1, variant=""):
    nc = bass.Bass()
    xh_in = nc.dram_tensor("xh", [P, XCOLS], F16, kind="ExternalInput")
    sc_in = nc.dram_tensor("sc", [P, NSC], F32, kind="ExternalInput")
    y_out = nc.dram_tensor("y", [P, CL], F16, kind="ExternalOutput")

    with _TC(nc) as tc:
        with (
            tc.tile_pool(name="main", bufs=1) as pool,
            tc.tile_pool(name="psum", bufs=2, space="PSUM") as psum_pool,
        ):
            xt = pool.tile([P, XCOLS], F16, tag="xt")   # x | diag blocks
            A = pool.tile([P, W], F16, tag="A")
            Bb = pool.tile([P, W], F16, tag="B")
            Cc = pool.tile([P, W], F16, tag="C")
            y = pool.tile([P, CL], F16, tag="y")
            E = pool.tile([P, CL], F16, tag="E")
            sct = pool.tile([P, NSC], F32, tag="sc")

            nc.sync.dma_start(out=sct[:], in_=sc_in[:])
            # diag blocks first (PE needs them immediately), then x stripes
            nc.sync.dma_start(out=xt[:, W:], in_=xh_in[:, W:])
            xb = [0, 2048, 4096, 6144, 8192, 10240, 12288, 14336, 16384, W]
            for a, b in zip(xb, xb[1:]):
                nc.sync.dma_start(out=xt[:, a:b], in_=xh_in[:, a:b])

            def dg(blk):
                return xt[:, W + blk * P:W + (blk + 1) * P]

            def rbuf(lvl):
                # chain buffer rotation: r0=xt(x region), then A,B,C,xt
                # cycling (x region is dead once G0's taps have read it).
                if lvl == 0:
                    return xt
                return [A, Bb, Cc, xt][(lvl - 1) % 4]

            def pe_group(taps, evict_to, gelu=False):
                """PE y-taps for one group over all stripes, evicting each
                stripe via ACT (Copy to E, or Gelu to evict_to)."""
                for s in range(NSTR):
                    ps = psum_pool.tile([P, STRIPE], F32, tag="ps")
                    for c in range(STRIPE // MMN):
                        o0 = PAD + s * STRIPE + c * MMN
                        for i, (lvl, k) in enumerate(taps):
                            if lvl == "w9":
                                src, sh = xt, 0
                            elif lvl == "w0":
                                src, sh = rbuf(7), k * 128
                            else:
                                src, sh = rbuf(lvl), k * (1 << lvl)
                            nc.tensor.matmul(
                                ps[:, c * MMN:(c + 1) * MMN],
                                lhsT=dg(_diag_block((lvl, k))),
                                rhs=src[:, o0 - sh:o0 - sh + MMN],
                                start=(i == 0), stop=(i == len(taps) - 1),
                            )
                    s0 = s * STRIPE
                    nc.scalar.activation(
                        out=evict_to[:, s0:s0 + STRIPE], in_=ps[:],
                        func=(mybir.ActivationFunctionType.Gelu if gelu
                              else mybir.ActivationFunctionType.Copy),
                    )
                    if gelu and reps == 1:
                        nc.sync.dma_start(
                            out=y_out[:, s0:s0 + STRIPE],
                            in_=evict_to[:, s0:s0 + STRIPE],
                        )

            def pe_chain_l0():
                """r1 = h0-conv(x, d=1) on PE, evicted to A (fp16)."""
                cb = [0, 2048, 4096, 6144, 8192, 10240, 12288, 14336, 16384,
                      W]
                for a, b in zip(cb, cb[1:]):
                    ps = psum_pool.tile([P, STRIPE], F32, tag="ps")
                    lo0 = max(a, 4)   # cols 0..3 read x[<0]; never consumed
                    for c0 in range(a, b, MMN):
                        c1 = min(c0 + MMN, b)
                        lo = max(c0, lo0)
                        for k in range(4):
                            nc.tensor.matmul(
                                ps[:, lo - a:c1 - a],
                                lhsT=dg(k),
                                rhs=xt[:, lo - k:c1 - k],
                                start=(k == 0), stop=(k == 3),
                            )
                    nc.scalar.copy(out=A[:, lo0:b], in_=ps[:, lo0 - a:b - a])

            def dve_chain(lvl):
                """r_{lvl+1} = h0-conv(r_lvl, d=2^lvl): ACT tap0 + 3 STT."""
                d = 1 << lvl
                lo = 6 * d - 3 + 1          # valid start, rounded even
                cur, nxt = rbuf(lvl), rbuf(lvl + 1)
                mid = (lo + W) // 2 // 2 * 2
                for a, b in ((lo, mid), (mid, W)):
                    nc.scalar.activation(
                        out=nxt[:, a:b], in_=cur[:, a:b],
                        func=mybir.ActivationFunctionType.Copy,
                        scale=sct[:, 3:4],
                    )
                for k in (1, 2, 3):
                    for a, b in ((lo, mid), (mid, W)):
                        nc.vector.scalar_tensor_tensor(
                            out=nxt[:, a:b],
                            in0=cur[:, a - k * d:b - k * d],
                            scalar=sct[:, k - 1:k],
                            in1=nxt[:, a:b],
                            op0=MULT, op1=ADD,
                        )

            def donate_tap(idx, create=False):
                """Donated y-tap on DVE; the first consumes E0 as in1."""
                lvl, k = DONATE[idx]
                d = 1 << lvl
                src = rbuf(lvl)
                segs = NSTR if create else 2
                for s in range(segs):
                    a = CL * s // segs
                    b = CL * (s + 1) // segs
                    nc.vector.scalar_tensor_tensor(
                        out=y[:, a:b],
                        in0=src[:, PAD + a - k * d:PAD + b - k * d],
                        scalar=sct[:, 4 + idx:5 + idx],
                        in1=(E if create else y)[:, a:b],
                        op0=MULT, op1=ADD,
                    )

            def merge():
                for s in range(NSTR):
                    a = s * STRIPE
                    nc.vector.tensor_tensor(
                        out=y[:, a:a + STRIPE], in0=y[:, a:a + STRIPE],
                        in1=E[:, a:a + STRIPE], op=ADD,
                    )

            for _rep in range(reps):
                # Emission order = per-engine execution order. The single E
                # buffer forces each merge/create (DVE) to be emitted before
                # the next group's evictions (ACT) so WAR deps bind right.
                pe_chain_l0()                 # PE: r1 -> A
                pe_group(_GROUPS[0], E)       # PE: E0 = w9*x + l0 y-taps
                dve_chain(1)                  # DVE: r2 -> B
                dve_chain(2)                  # DVE: r3 -> C
                donate_tap(0, create=True)    # DVE: y = s*r2 + E0
                pe_group(_GROUPS[1], E)       # PE: E1 = l1 + l2 taps
                dve_chain(3)                  # DVE: r4 -> xt
                merge()                       # DVE: y += E1
                pe_group(_GROUPS[2], E)       # PE: E2 = l3 + l4 taps
                dve_chain(4)                  # DVE: r5 -> A
                donate_tap(1)                 # DVE: y += s*r3
                merge()                       # DVE: y += E2
                dve_chain(5)                  # DVE: r6 -> B
                pe_group(_GROUPS[3], E)       # PE: E3 = l5 + l6 taps
                dve_chain(6)                  # DVE: r7 -> C
                merge()                       # DVE: y += E3
                pe_group(_GROUPS[4], E)       # PE: E4 = l7 + w0 taps
                donate_tap(2)                 # DVE: y += s*r4
                donate_tap(3)                 # DVE: y += s*r5
                donate_tap(4)                 # DVE: y += s*r6
                merge()                       # DVE: y += E4
                # --- final gelu + out DMA, streamed per stripe ---
                for s in range(NSTR):
                    a = s * STRIPE
                    nc.scalar.activation(
                        out=y[:, a:a + STRIPE], in_=y[:, a:a + STRIPE],
                        func=mybir.ActivationFunctionType.Gelu,
                    )
                    if _rep == reps - 1:
                        nc.sync.dma_start(
                            out=y_out[:, a:a + STRIPE], in_=y[:, a:a + STRIPE]
                        )
    return nc


_NC_CACHE = {}


def _get_nc(reps=1, variant=""):
    key = (reps, variant)
    if key not in _NC_CACHE:
        _NC_CACHE[key] = _build_nc(reps, variant)
    return _NC_CACHE[key]


def _tap_scalar(h0, h1, w, tap):
    """per-channel weight vector [64] for a tap."""
    lvl, k = tap
    if lvl == "w9":
        return w[:, DEPTH + 1]
    if lvl == "w0":
        return w[:, 0] * h0[:, 0, K - 1 - k]
    return w[:, DEPTH - lvl] * h1[:, 0, K - 1 - k]


def pack_inputs(x, h0, h1, w):
    """Host-side packing into per-core fp16 [P, XCOLS] + fp32 [P, NSC]."""
    diag = np.zeros((P, NDIAG * P), np.float16)

    def set_diag(blk, v64):
        v = np.tile(v64, NCHUNK)
        diag[np.arange(P), blk * P + np.arange(P)] = v.astype(np.float16)

    for k in range(4):
        set_diag(k, h0[:, 0, K - 1 - k])          # chain blocks
    set_diag(4, w[:, DEPTH + 1])                  # w9
    for i, tap in enumerate(_PE_Y):
        set_diag(5 + i, _tap_scalar(h0, h1, w, tap))

    sc = np.zeros((C, NSC), np.float32)
    for k in (1, 2, 3):
        sc[:, k - 1] = h0[:, 0, K - 1 - k]
    sc[:, 3] = h0[:, 0, K - 1]
    for i, tap in enumerate(DONATE):
        sc[:, 4 + i] = _tap_scalar(h0, h1, w, tap)
    sc = np.tile(sc, (NCHUNK, 1))

    x16 = np.asarray(x, np.float16)
    in_maps = []
    for b in range(NCORES):
        buf = np.zeros((P, XCOLS), np.float16)
        for j in range(NCHUNK):
            lo = j * CL
            if lo >= PAD:
                buf[j * C:(j + 1) * C, :W] = x16[b, :, lo - PAD:lo + CL]
            else:
                buf[j * C:(j + 1) * C, PAD:W] = x16[b, :, lo:lo + CL]
        buf[:, W:] = diag
        in_maps.append({"xh": buf, "sc": sc})
    return in_maps


def unpack_outputs(results):
    out = np.empty((B, C, L), np.float32)
    for b, r in enumerate(results):
        yv = np.asarray(r["y"], np.float32)
        for j in range(NCHUNK):
            out[b, :, j * CL:(j + 1) * CL] = yv[j * C:(j + 1) * C]
    return out


def kernel(x, h0, h1, w, _trace=False, _variant=""):
    import os
    os.environ.setdefault("BASS_NEVER_TRACE", "1")

    x = np.asarray(x, np.float32)
    h0 = np.asarray(h0, np.float32)
    h1 = np.asarray(h1, np.float32)
    w = np.asarray(w, np.float32)

    in_maps = pack_inputs(x, h0, h1, w)
    nc = _get_nc(1, _variant)
    try:
        res = run_bass_kernel_spmd(
            nc, in_maps, core_ids=list(range(NCORES)), trace=_trace,
        )
    except Exception:
        res = run_bass_kernel_spmd(
            nc, in_maps, core_ids=list(range(NCORES)), trace=_trace,
        )
    out = unpack_outputs(res.results)
    if _trace:
        return out, res
    return out



# revision 3
# speedup vs baseline: 1.3036x; 1.3036x over previous
"""Trainium2 Bass kernel for nn_CausalMultiresConv1d (composite-FIR matmul).

The whole module is, per channel c, one causal FIR filter:
    y = gelu(F_c (*) x_c),   F_c = w9 d + sum_lvl w_{8-lvl} (h1_lvl (*) H0_lvl)
                                   + w0 H0_8        (766 taps, built on host)
where H0_lvl is the composition of the first lvl dilated h0 convs.

Device layout: per channel, the signal is L-major across partitions
(l = 128*f + p  ->  tile [128 parts, 256 cols], plus 6 left zero-pad cols).
A shift by s = 128*j + r then factors into a column shift j plus a
partition shift r, so the full 766-tap conv is SEVEN matmuls per channel:
    M_j[pi, po] = F_c[128*j + po - pi]   (Toeplitz band, j = 0..6)
    psum += M_j^T @ x[:, 6-j : 262-j]
accumulated in one PSUM tile, evicted once by ACT with exact-erf Gelu.

Sharding: pure data parallel - 1 batch element per NeuronCore (B=8).
Everything lands on PE (~48us engine time); DVE is idle, ACT only runs
the gelu evictions. Stationaries (14.7MB fp16) + x (4.3MB) stream in
interleaved per 4-channel group so matmuls start after the first group.
"""

import numpy as np

import concourse.bass as bass
import concourse.mybir as mybir
from concourse.bass_utils import run_bass_kernel_spmd
from concourse.tile import TileContext

# The walrus build here rejects instructions carrying more than one sync-wait
# ("Too many sync wait commands"). Tile's kernel-tail drain attaches a wait
# for every outstanding semaphore to a single SP Drain. _TC splits them.


class _TC(TileContext):
    def __exit__(self, *a):
        r = super().__exit__(*a)
        _split_multi_waits(self.nc)
        return r


def _split_multi_waits(nc):
    n = 0
    for fn in nc.m.functions:
        for blk in fn.blocks:
            insts = getattr(blk, "instructions", None)
            if insts is None:
                continue
            new = []
            for inst in insts:
                si = getattr(inst, "sync_info", None)
                waits = list(si.on_wait) if si is not None and si.on_wait else []
                if len(waits) > 1:
                    for j, wcmd in enumerate(waits[:-1]):
                        nop = mybir.InstNoOp(
                            name=f"{inst.name}-hw{j}", engine=inst.engine
                        )
                        nop.sync_info = mybir.SyncInfo(
                            on_wait=[wcmd], on_update=[]
                        )
                        new.append(nop)
                        n += 1
                    inst.sync_info = mybir.SyncInfo(
                        on_wait=[waits[-1]], on_update=list(si.on_update)
                    )
                new.append(inst)
            blk.instructions[:] = new
    return n


B, C, L = 8, 64, 32768
K, DEPTH = 4, 8
NCORES = 8
P = 128                   # partitions; l = 128*f + p within a channel
FREE = L // P             # 256 cols per channel
NJ = 7                    # ceil(766/128): stationary band matrices per chan
PADC = NJ - 1             # left zero-pad cols (6*128 = 768 >= 765 taps)
CW = PADC + FREE          # 262 x-cols per channel
NTAPS = 766               # composite filter support
GRP = 4                   # channels per PSUM tile / DMA chunk

XCOLS = C * CW            # 16768
SCOLS = C * NJ * P        # 57344
YCOLS = C * FREE          # 16384

F16 = mybir.dt.float16
F32 = mybir.dt.float32


def _build_nc(reps=1, variant=""):
    nc = bass.Bass()
    xh_in = nc.dram_tensor("xh", [P, XCOLS], F16, kind="ExternalInput")
    st_in = nc.dram_tensor("st", [P, SCOLS], F16, kind="ExternalInput")
    y_out = nc.dram_tensor("y", [P, YCOLS], F16, kind="ExternalOutput")

    with _TC(nc) as tc:
        with (
            tc.tile_pool(name="main", bufs=1) as pool,
            tc.tile_pool(name="psum", bufs=2, space="PSUM") as psum_pool,
        ):
            xt = pool.tile([P, XCOLS], F16, tag="xt")
            st = pool.tile([P, SCOLS], F16, tag="st")
            yt = pool.tile([P, YCOLS], F16, tag="yt")

            # interleave stationary/x chunks per channel group so group 0's
            # matmuls start as soon as its operands land
            for g in range(C // GRP):
                a = g * GRP
                nc.sync.dma_start(
                    out=st[:, a * NJ * P:(a + GRP) * NJ * P],
                    in_=st_in[:, a * NJ * P:(a + GRP) * NJ * P],
                )
                nc.sync.dma_start(
                    out=xt[:, a * CW:(a + GRP) * CW],
                    in_=xh_in[:, a * CW:(a + GRP) * CW],
                )

            for _rep in range(reps):
                for g in range(C // GRP):
                    ps = psum_pool.tile([P, GRP * FREE], F32, tag="ps")
                    for ci in range(GRP):
                        c = g * GRP + ci
                        for j in range(NJ):
                            nc.tensor.matmul(
                                ps[:, ci * FREE:(ci + 1) * FREE],
                                lhsT=st[:, (c * NJ + j) * P:(c * NJ + j + 1) * P],
                                rhs=xt[:, c * CW + PADC - j:c * CW + PADC - j + FREE],
                                start=(j == 0), stop=(j == NJ - 1),
                            )
                    a = g * GRP * FREE
                    nc.scalar.activation(
                        out=yt[:, a:a + GRP * FREE], in_=ps[:],
                        func=mybir.ActivationFunctionType.Gelu,
                    )
                    if _rep == reps - 1:
                        nc.sync.dma_start(
                            out=y_out[:, a:a + GRP * FREE],
                            in_=yt[:, a:a + GRP * FREE],
                        )
    return nc


_NC_CACHE = {}


def _get_nc(reps=1, variant=""):
    key = (reps, variant)
    if key not in _NC_CACHE:
        _NC_CACHE[key] = _build_nc(reps, variant)
    return _NC_CACHE[key]


def _composite_filter(h0, h1, w):
    """F [C, NTAPS] float64: per-channel composite causal FIR."""

    def dil(g, d):
        out = np.zeros((len(g) - 1) * d + 1)
        out[::d] = g
        return out

    F = np.zeros((C, NTAPS))
    for c in range(C):
        g0 = h0[c, 0, ::-1].astype(np.float64)
        g1 = h1[c, 0, ::-1].astype(np.float64)
        G = np.array([1.0])
        d = 1
        for i in range(DEPTH, 0, -1):
            hi = np.convolve(dil(g1, d), G)
            F[c, :len(hi)] += w[c, i] * hi
            G = np.convolve(dil(g0, d), G)
            d *= 2
        F[c, :len(G)] += w[c, 0] * G
        F[c, 0] += w[c, DEPTH + 1]
    return F


def pack_inputs(x, h0, h1, w):
    """Host-side packing: per-core fp16 x tiles + shared stationary table."""
    F = _composite_filter(h0, h1, w).astype(np.float16)

    # st[pi, (c*NJ + j)*P + po] = F[c, 128*j + po - pi] (0 outside [0,765])
    pi = np.arange(P)[:, None]
    po = np.arange(P)[None, :]
    st = np.zeros((P, SCOLS), np.float16)
    for j in range(NJ):
        idx = 128 * j + po - pi            # [P, P]
        valid = (idx >= 0) & (idx < NTAPS)
        idxc = np.clip(idx, 0, NTAPS - 1)
        blk = np.where(valid[None], F[:, idxc], 0)   # [C, P, P]
        for c in range(C):
            st[:, (c * NJ + j) * P:(c * NJ + j + 1) * P] = blk[c]

    x16 = np.asarray(x, np.float16)
    in_maps = []
    for b in range(NCORES):
        buf = np.zeros((C, P, CW), np.float16)
        buf[:, :, PADC:] = x16[b].reshape(C, FREE, P).transpose(0, 2, 1)
        in_maps.append(
            {"xh": np.ascontiguousarray(buf.transpose(1, 0, 2)).reshape(P, XCOLS),
             "st": st}
        )
    return in_maps


def unpack_outputs(results):
    out = np.empty((B, C, L), np.float32)
    for b, r in enumerate(results):
        yv = np.asarray(r["y"], np.float32)          # [P, C*FREE]
        out[b] = yv.reshape(P, C, FREE).transpose(1, 2, 0).reshape(C, L)
    return out


def kernel(x, h0, h1, w, _trace=False, _variant=""):
    import os
    os.environ.setdefault("BASS_NEVER_TRACE", "1")

    x = np.asarray(x, np.float32)
    h0 = np.asarray(h0, np.float32)
    h1 = np.asarray(h1, np.float32)
    w = np.asarray(w, np.float32)

    in_maps = pack_inputs(x, h0, h1, w)
    nc = _get_nc(1, _variant)
    try:
        res = run_bass_kernel_spmd(
            nc, in_maps, core_ids=list(range(NCORES)), trace=_trace,
        )
    except Exception:
        res = run_bass_kernel_spmd(
            nc, in_maps, core_ids=list(range(NCORES)), trace=_trace,
        )
    out = unpack_outputs(res.results)
    if _trace:
        return out, res
    return out


# revision 5
# speedup vs baseline: 6.7285x; 5.1616x over previous
"""Trainium2 Bass kernel for nn_CausalMultiresConv1d (composite-FIR matmul).

The whole module is, per channel c, one causal FIR filter:
    y = gelu(F_c (*) x_c),   F_c = w9 d + sum_lvl w_{8-lvl} (h1_lvl (*) H0_lvl)
                                   + w0 H0_8        (766 taps, built on host)
where H0_lvl is the composition of the first lvl dilated h0 convs.

Device layout: per channel, the signal is L-major across partitions
(l = 128*f + p  ->  tile [128 parts, 256 cols], plus 6 left zero-pad cols).
A shift by s = 128*j + r then factors into a column shift j plus a
partition shift r, so the full 766-tap conv is SEVEN matmuls per channel:
    M_j[pi, po] = F_c[128*j + po - pi]   (Toeplitz band, j = 0..6)
    psum += M_j^T @ x[:, 6-j : 262-j]
accumulated in one PSUM tile, evicted once by ACT with exact-erf Gelu.

Sharding: pure data parallel - 1 batch element per NeuronCore (B=8).
Everything lands on PE (~48us engine time); DVE is idle, ACT only runs
the gelu evictions. Stationaries (14.7MB fp16) + x (4.3MB) stream in
interleaved per 4-channel group so matmuls start after the first group.
"""

import numpy as np

import concourse.bass as bass
import concourse.mybir as mybir
from concourse.bass_utils import run_bass_kernel_spmd
from concourse.tile import TileContext

# The walrus build here rejects instructions carrying more than one sync-wait
# ("Too many sync wait commands"). Tile's kernel-tail drain attaches a wait
# for every outstanding semaphore to a single SP Drain. _TC splits them.


class _TC(TileContext):
    def __exit__(self, *a):
        r = super().__exit__(*a)
        _split_multi_waits(self.nc)
        return r


def _split_multi_waits(nc):
    n = 0
    for fn in nc.m.functions:
        for blk in fn.blocks:
            insts = getattr(blk, "instructions", None)
            if insts is None:
                continue
            new = []
            for inst in insts:
                si = getattr(inst, "sync_info", None)
                waits = list(si.on_wait) if si is not None and si.on_wait else []
                if len(waits) > 1:
                    for j, wcmd in enumerate(waits[:-1]):
                        nop = mybir.InstNoOp(
                            name=f"{inst.name}-hw{j}", engine=inst.engine
                        )
                        nop.sync_info = mybir.SyncInfo(
                            on_wait=[wcmd], on_update=[]
                        )
                        new.append(nop)
                        n += 1
                    inst.sync_info = mybir.SyncInfo(
                        on_wait=[waits[-1]], on_update=list(si.on_update)
                    )
                new.append(inst)
            blk.instructions[:] = new
    return n


B, C, L = 8, 64, 32768
K, DEPTH = 4, 8
NCORES = 8
P = 128                   # partitions; l = 128*f + p within a channel
FREE = L // P             # 256 cols per channel
NJ = 7                    # ceil(766/128): stationary band matrices per chan
PADC = NJ - 1             # left zero-pad cols (6*128 = 768 >= 765 taps)
CW = PADC + FREE          # 262 x-cols per channel
NTAPS = 766               # composite filter support
GRP = 4                   # channels per PSUM tile / DMA chunk

XCOLS = C * CW            # 16768
SCOLS = C * NJ * P        # 57344
YCOLS = C * FREE          # 16384

F16 = mybir.dt.float16
F32 = mybir.dt.float32


def _build_nc(reps=1, variant=""):
    grp = GRP
    if variant.startswith("g"):
        grp = int(variant[1:])
    nc = bass.Bass()
    xh_in = nc.dram_tensor("xh", [P, XCOLS], F16, kind="ExternalInput")
    st_in = nc.dram_tensor("st", [P, SCOLS], F16, kind="ExternalInput")
    y_out = nc.dram_tensor("y", [P, YCOLS], F16, kind="ExternalOutput")

    with _TC(nc) as tc:
        with (
            tc.tile_pool(name="main", bufs=1) as pool,
            tc.tile_pool(name="psum", bufs=2, space="PSUM") as psum_pool,
        ):
            xt = pool.tile([P, XCOLS], F16, tag="xt")
            st = pool.tile([P, SCOLS], F16, tag="st")
            yt = pool.tile([P, YCOLS], F16, tag="yt")

            # interleave stationary/x chunks per channel group so group 0's
            # matmuls start as soon as its operands land
            for g in range(C // 4):
                a = g * 4
                nc.sync.dma_start(
                    out=st[:, a * NJ * P:(a + 4) * NJ * P],
                    in_=st_in[:, a * NJ * P:(a + 4) * NJ * P],
                )
                nc.sync.dma_start(
                    out=xt[:, a * CW:(a + 4) * CW],
                    in_=xh_in[:, a * CW:(a + 4) * CW],
                )

            for _rep in range(reps):
                for g in range(C // grp):
                    ps = psum_pool.tile([P, grp * FREE], F32, tag="ps")
                    for ci in range(grp):
                        c = g * grp + ci
                        for j in range(NJ):
                            nc.tensor.matmul(
                                ps[:, ci * FREE:(ci + 1) * FREE],
                                lhsT=st[:, (c * NJ + j) * P:(c * NJ + j + 1) * P],
                                rhs=xt[:, c * CW + PADC - j:c * CW + PADC - j + FREE],
                                start=(j == 0), stop=(j == NJ - 1),
                            )
                    a = g * grp * FREE
                    nc.scalar.activation(
                        out=yt[:, a:a + grp * FREE], in_=ps[:],
                        func=mybir.ActivationFunctionType.Gelu,
                    )
                    if _rep == reps - 1:
                        nc.sync.dma_start(
                            out=y_out[:, a:a + grp * FREE],
                            in_=yt[:, a:a + grp * FREE],
                        )
    return nc


_NC_CACHE = {}


def _get_nc(reps=1, variant=""):
    key = (reps, variant)
    if key not in _NC_CACHE:
        _NC_CACHE[key] = _build_nc(reps, variant)
    return _NC_CACHE[key]


def _composite_filter(h0, h1, w):
    """F [C, NTAPS] float64: per-channel composite causal FIR."""

    def dil(g, d):
        out = np.zeros((len(g) - 1) * d + 1)
        out[::d] = g
        return out

    F = np.zeros((C, NTAPS))
    for c in range(C):
        g0 = h0[c, 0, ::-1].astype(np.float64)
        g1 = h1[c, 0, ::-1].astype(np.float64)
        G = np.array([1.0])
        d = 1
        for i in range(DEPTH, 0, -1):
            hi = np.convolve(dil(g1, d), G)
            F[c, :len(hi)] += w[c, i] * hi
            G = np.convolve(dil(g0, d), G)
            d *= 2
        F[c, :len(G)] += w[c, 0] * G
        F[c, 0] += w[c, DEPTH + 1]
    return F


def pack_inputs(x, h0, h1, w):
    """Host-side packing: per-core fp16 x tiles + shared stationary table."""
    F = _composite_filter(h0, h1, w).astype(np.float16)

    # st[pi, (c*NJ + j)*P + po] = F[c, 128*j + po - pi] (0 outside [0,765])
    pi = np.arange(P)[:, None]
    po = np.arange(P)[None, :]
    st = np.zeros((P, SCOLS), np.float16)
    for j in range(NJ):
        idx = 128 * j + po - pi            # [P, P]
        valid = (idx >= 0) & (idx < NTAPS)
        idxc = np.clip(idx, 0, NTAPS - 1)
        blk = np.where(valid[None], F[:, idxc], 0)   # [C, P, P]
        for c in range(C):
            st[:, (c * NJ + j) * P:(c * NJ + j + 1) * P] = blk[c]

    x16 = np.asarray(x, np.float16)
    in_maps = []
    for b in range(NCORES):
        buf = np.zeros((C, P, CW), np.float16)
        buf[:, :, PADC:] = x16[b].reshape(C, FREE, P).transpose(0, 2, 1)
        in_maps.append(
            {"xh": np.ascontiguousarray(buf.transpose(1, 0, 2)).reshape(P, XCOLS),
             "st": st}
        )
    return in_maps


def unpack_outputs(results):
    out = np.empty((B, C, L), np.float32)
    for b, r in enumerate(results):
        yv = np.asarray(r["y"], np.float32)          # [P, C*FREE]
        out[b] = yv.reshape(P, C, FREE).transpose(1, 2, 0).reshape(C, L)
    return out


def kernel(x, h0, h1, w, _trace=False, _variant=""):
    import os
    os.environ.setdefault("BASS_NEVER_TRACE", "1")

    x = np.asarray(x, np.float32)
    h0 = np.asarray(h0, np.float32)
    h1 = np.asarray(h1, np.float32)
    w = np.asarray(w, np.float32)

    in_maps = pack_inputs(x, h0, h1, w)
    nc = _get_nc(1, _variant)
    try:
        res = run_bass_kernel_spmd(
            nc, in_maps, core_ids=list(range(NCORES)), trace=_trace,
        )
    except Exception:
        res = run_bass_kernel_spmd(
            nc, in_maps, core_ids=list(range(NCORES)), trace=_trace,
        )
    out = unpack_outputs(res.results)
    if _trace:
        return out, res
    return out


# revision 7
# speedup vs baseline: 7.1490x; 1.0625x over previous
"""Trainium2 Bass kernel for nn_CausalMultiresConv1d (composite-FIR matmul).

The whole module is, per channel c, one causal FIR filter:
    y = gelu(F_c (*) x_c),   F_c = w9 d + sum_lvl w_{8-lvl} (h1_lvl (*) H0_lvl)
                                   + w0 H0_8        (766 taps, built on host)
where H0_lvl is the composition of the first lvl dilated h0 convs.

Device layout: per channel, the signal is L-major across partitions
(l = 128*f + p  ->  tile [128 parts, 256 cols], plus 6 left zero-pad cols).
A shift by s = 128*j + r then factors into a column shift j plus a
partition shift r, so the full 766-tap conv is SEVEN matmuls per channel:
    M_j[pi, po] = F_c[128*j + po - pi]   (Toeplitz band, j = 0..6)
    psum += M_j^T @ x[:, 6-j : 262-j]
j = 0..1 run in fp16 into psA; j = 2..6 (tail, ~9% of y energy) run with
fp8e4m3 stationaries scaled per-channel by 2^k_c into psB (fp16 rhs -
mixed-dtype matmul). DVE recombines yt = 2^-k_c * psB + psA per channel,
ACT applies exact-erf Gelu in place. Rel err ~7e-3 (tolerance 2e-2).

Sharding: pure data parallel - 1 batch element per NeuronCore (B=8).
PE does all the conv math (~55us/rep); DVE only the 64 recombines, ACT
only the geluss. Stationaries (9.5MB) + x (4.3MB) stream in per
4-channel group so matmuls start after the first group lands.
"""

import numpy as np

import concourse.bass as bass
import concourse.mybir as mybir
from concourse.bass_utils import run_bass_kernel_spmd
from concourse.tile import TileContext

# The walrus build here rejects instructions carrying more than one sync-wait
# ("Too many sync wait commands"). Tile's kernel-tail drain attaches a wait
# for every outstanding semaphore to a single SP Drain. _TC splits them.


class _TC(TileContext):
    def __exit__(self, *a):
        r = super().__exit__(*a)
        _split_multi_waits(self.nc)
        return r


def _split_multi_waits(nc):
    n = 0
    for fn in nc.m.functions:
        for blk in fn.blocks:
            insts = getattr(blk, "instructions", None)
            if insts is None:
                continue
            new = []
            for inst in insts:
                si = getattr(inst, "sync_info", None)
                waits = list(si.on_wait) if si is not None and si.on_wait else []
                if len(waits) > 1:
                    for j, wcmd in enumerate(waits[:-1]):
                        nop = mybir.InstNoOp(
                            name=f"{inst.name}-hw{j}", engine=inst.engine
                        )
                        nop.sync_info = mybir.SyncInfo(
                            on_wait=[wcmd], on_update=[]
                        )
                        new.append(nop)
                        n += 1
                    inst.sync_info = mybir.SyncInfo(
                        on_wait=[waits[-1]], on_update=list(si.on_update)
                    )
                new.append(inst)
            blk.instructions[:] = new
    return n


B, C, L = 8, 64, 32768
K, DEPTH = 4, 8
NCORES = 8
P = 128                   # partitions; l = 128*f + p within a channel
FREE = L // P             # 256 cols per channel
NJ = 7                    # ceil(766/128): stationary band matrices per chan
JCUT = 2                  # j < JCUT fp16, j >= JCUT scaled fp8
NJ8 = NJ - JCUT
PADC = NJ - 1             # left zero-pad cols (6*128 = 768 >= 765 taps)
CW = PADC + FREE          # 262 x-cols per channel
NTAPS = 766               # composite filter support
GRP = 4                   # channels per PSUM tile / DMA chunk

XCOLS = C * CW            # 16768
SACOLS = C * JCUT * P     # fp16 stationary cols
SBCOLS = C * NJ8 * P      # fp8 stationary cols
YCOLS = C * FREE          # 16384

F16 = mybir.dt.float16
F32 = mybir.dt.float32
F8 = mybir.dt.float8e4
MULT = mybir.AluOpType.mult
ADD = mybir.AluOpType.add


def _build_nc(reps=1, variant=""):
    grp = GRP
    if variant.startswith("g"):
        grp = int(variant[1:])
    nc = bass.Bass()
    xh_in = nc.dram_tensor("xh", [P, XCOLS], F16, kind="ExternalInput")
    sa_in = nc.dram_tensor("sa", [P, SACOLS], F16, kind="ExternalInput")
    sb_in = nc.dram_tensor("sb", [P, SBCOLS], F8, kind="ExternalInput")
    sc_in = nc.dram_tensor("sc", [P, C], F32, kind="ExternalInput")
    y_out = nc.dram_tensor("y", [P, YCOLS], F16, kind="ExternalOutput")

    with _TC(nc) as tc:
        with (
            tc.tile_pool(name="main", bufs=1) as pool,
            tc.tile_pool(name="psum", bufs=2, space="PSUM") as psum_pool,
        ):
            xt = pool.tile([P, XCOLS], F16, tag="xt")
            sa = pool.tile([P, SACOLS], F16, tag="sa")
            sb = pool.tile([P, SBCOLS], F8, tag="sb")
            sct = pool.tile([P, C], F32, tag="sc")
            yt = pool.tile([P, YCOLS], F16, tag="yt")

            nc.sync.dma_start(out=sct[:], in_=sc_in[:])
            # interleave stationary/x chunks per 4 channels so group 0's
            # matmuls start as soon as its operands land
            for g in range(C // 4):
                a = g * 4
                nc.sync.dma_start(
                    out=sa[:, a * JCUT * P:(a + 4) * JCUT * P],
                    in_=sa_in[:, a * JCUT * P:(a + 4) * JCUT * P],
                )
                nc.sync.dma_start(
                    out=sb[:, a * NJ8 * P:(a + 4) * NJ8 * P],
                    in_=sb_in[:, a * NJ8 * P:(a + 4) * NJ8 * P],
                )
                nc.sync.dma_start(
                    out=xt[:, a * CW:(a + 4) * CW],
                    in_=xh_in[:, a * CW:(a + 4) * CW],
                )

            for _rep in range(reps):
                for g in range(C // grp):
                    psA = psum_pool.tile([P, grp * FREE], F32, tag="psA")
                    psB = psum_pool.tile([P, grp * FREE], F32, tag="psB")
                    for ci in range(grp):
                        c = g * grp + ci
                        rh = xt[:, c * CW + PADC:c * CW + PADC + FREE]
                        for j in range(JCUT):
                            nc.tensor.matmul(
                                psA[:, ci * FREE:(ci + 1) * FREE],
                                lhsT=sa[:, (c * JCUT + j) * P:
                                         (c * JCUT + j + 1) * P],
                                rhs=xt[:, c * CW + PADC - j:
                                       c * CW + PADC - j + FREE],
                                start=(j == 0), stop=(j == JCUT - 1),
                            )
                        for j8 in range(NJ8):
                            j = JCUT + j8
                            nc.tensor.matmul(
                                psB[:, ci * FREE:(ci + 1) * FREE],
                                lhsT=sb[:, (c * NJ8 + j8) * P:
                                         (c * NJ8 + j8 + 1) * P],
                                rhs=xt[:, c * CW + PADC - j:
                                       c * CW + PADC - j + FREE],
                                start=(j8 == 0), stop=(j8 == NJ8 - 1),
                            )
                    a = g * grp * FREE
                    nc.scalar.activation(
                        out=yt[:, a:a + grp * FREE], in_=psA[:],
                        func=mybir.ActivationFunctionType.Copy,
                    )
                    for ci in range(grp):
                        c = g * grp + ci
                        sl = slice(ci * FREE, (ci + 1) * FREE)
                        nc.vector.scalar_tensor_tensor(
                            out=yt[:, c * FREE:(c + 1) * FREE],
                            in0=psB[:, sl],
                            scalar=sct[:, c:c + 1],
                            in1=yt[:, c * FREE:(c + 1) * FREE],
                            op0=MULT, op1=ADD,
                        )
                    nc.scalar.activation(
                        out=yt[:, a:a + grp * FREE],
                        in_=yt[:, a:a + grp * FREE],
                        func=mybir.ActivationFunctionType.Gelu,
                    )
                    if _rep == reps - 1:
                        nc.sync.dma_start(
                            out=y_out[:, a:a + grp * FREE],
                            in_=yt[:, a:a + grp * FREE],
                        )
    return nc


_NC_CACHE = {}


def _get_nc(reps=1, variant=""):
    key = (reps, variant)
    if key not in _NC_CACHE:
        _NC_CACHE[key] = _build_nc(reps, variant)
    return _NC_CACHE[key]


def _composite_filter(h0, h1, w):
    """F [C, NTAPS] float64: per-channel composite causal FIR."""

    def dil(g, d):
        out = np.zeros((len(g) - 1) * d + 1)
        out[::d] = g
        return out

    F = np.zeros((C, NTAPS))
    for c in range(C):
        g0 = h0[c, 0, ::-1].astype(np.float64)
        g1 = h1[c, 0, ::-1].astype(np.float64)
        G = np.array([1.0])
        d = 1
        for i in range(DEPTH, 0, -1):
            hi = np.convolve(dil(g1, d), G)
            F[c, :len(hi)] += w[c, i] * hi
            G = np.convolve(dil(g0, d), G)
            d *= 2
        F[c, :len(G)] += w[c, 0] * G
        F[c, 0] += w[c, DEPTH + 1]
    return F


def pack_inputs(x, h0, h1, w):
    """Host-side packing: per-core fp16 x tiles + shared stationary tables."""
    F = _composite_filter(h0, h1, w)
    np8 = mybir.dt.np(F8)

    # per-channel scale for the fp8 tail: max|entry| -> ~192 (e4m3 safe)
    scut = 128 * JCUT - 127
    kexp = np.zeros(C)
    for c in range(C):
        m = np.abs(F[c, scut:]).max()
        if m > 0:
            kexp[c] = np.floor(np.log2(192.0 / m))
    sc = np.tile((2.0 ** -kexp).astype(np.float32), (P, 1))

    # sa[pi, (c*JCUT + j)*P + po] = F[c, 128*j + po - pi]   (fp16, j < JCUT)
    # sb[pi, (c*NJ8 + j8)*P + po] = F[c, ...] * 2^k_c       (fp8, j >= JCUT)
    pi = np.arange(P)[:, None]
    po = np.arange(P)[None, :]
    sa = np.zeros((P, SACOLS), np.float16)
    sb = np.zeros((P, SBCOLS), np8)
    for j in range(NJ):
        idx = 128 * j + po - pi            # [P, P]
        valid = (idx >= 0) & (idx < NTAPS)
        idxc = np.clip(idx, 0, NTAPS - 1)
        blk = np.where(valid[None], F[:, idxc], 0)   # [C, P, P]
        for c in range(C):
            if j < JCUT:
                sa[:, (c * JCUT + j) * P:(c * JCUT + j + 1) * P] = (
                    blk[c].astype(np.float16))
            else:
                j8 = j - JCUT
                sb[:, (c * NJ8 + j8) * P:(c * NJ8 + j8 + 1) * P] = (
                    (blk[c] * 2.0 ** kexp[c]).astype(np8))

    x16 = np.asarray(x, np.float16)
    in_maps = []
    for b in range(NCORES):
        buf = np.zeros((C, P, CW), np.float16)
        buf[:, :, PADC:] = x16[b].reshape(C, FREE, P).transpose(0, 2, 1)
        in_maps.append(
            {"xh": np.ascontiguousarray(buf.transpose(1, 0, 2)).reshape(P, XCOLS),
             "sa": sa, "sb": sb, "sc": sc}
        )
    return in_maps


def unpack_outputs(results):
    out = np.empty((B, C, L), np.float32)
    for b, r in enumerate(results):
        yv = np.asarray(r["y"], np.float32)          # [P, C*FREE]
        out[b] = yv.reshape(P, C, FREE).transpose(1, 2, 0).reshape(C, L)
    return out


def kernel(x, h0, h1, w, _trace=False, _variant=""):
    import os
    os.environ.setdefault("BASS_NEVER_TRACE", "1")

    x = np.asarray(x, np.float32)
    h0 = np.asarray(h0, np.float32)
    h1 = np.asarray(h1, np.float32)
    w = np.asarray(w, np.float32)

    in_maps = pack_inputs(x, h0, h1, w)
    nc = _get_nc(1, _variant)
    try:
        res = run_bass_kernel_spmd(
            nc, in_maps, core_ids=list(range(NCORES)), trace=_trace,
        )
    except Exception:
        res = run_bass_kernel_spmd(
            nc, in_maps, core_ids=list(range(NCORES)), trace=_trace,
        )
    out = unpack_outputs(res.results)
    if _trace:
        return out, res
    return out
